# revision 13
# baseline (speedup 1.0000x reference)
"""Trainium2 Bass kernel for CartesianDecomposedAttention (complex-valued attention).

Reference math (complex):
  Q = (x @ wq.T) * rotor ; K = (x @ wk.T) * rotor ; V = x @ wv.T
  scores = Q conj(K)^T / sqrt(Dh)
  attn_w = softmax(scores.re) * exp(i * scores.im)
  out    = (attn_w @ V) @ wo.T        -> stack([re, im], -1)

Sharding over 8 cores: core c -> batch b=c//4, head-group g=c%4 (4 heads, 256
model dims per group). Each core computes a partial output [S, D] (re+im);
the host sums the 4 group partials per batch (no on-device collectives).

Device decomposition (matmul operands fp16, PSUM accumulation fp32,
everything transposed so no on-chip transposes are needed):
  - Host passes x^T, w^T slices (fp16), and negated copies where a
    PSUM-accumulation subtraction is needed (PSUM can only add).
  - RoPE via host-built cos/sin tables [128, S] (fp32); the 1/sqrt(Dh) score
    scale is folded into the Q-side tables. Stored components are chosen so
    both scoresT.re and scoresT.im are pure PSUM additions:
      Q: (Qr, Qin=-Qi)   K: (Kr, Kin=-Ki, Krn=-Kr)   V: (Vin=-Vi, Vr, Vi)
      Sr  = Kr.T Qr + Kin.T Qin          (= scores.re)
      Sip = Krn.T Qin + Kin.T Qr         (= +scores.im)
      attn_re = Vr.T ar + Vin.T ai ; attn_im = Vi.T ar + Vr.T ai
  - softmax without max-subtraction (scores in [-8, 8]); the denominator is a
    ones-vector matmul over exp tiles, applied *after* the AV matmul
    (1/r via reciprocal_approx_fast on a gpsimd partition_broadcast tile).
  - HW Sin is valid only on [-pi, pi]: two add_range_wraps straight from the
    scores.im PSUM tile (shift 0 -> sin arg, shift pi/2 -> cos arg; inputs
    stay within one 2pi period of [-pi, pi]).
  - ACT exp and sin live in different table sets (~2.7us per switch): chunks
    are processed in groups of two with all exps batched before all sins,
    enforced with explicit scheduling deps.
"""

import sys

for _p in ("/opt/trn_rl_repo",):
    if _p not in sys.path:
        sys.path.insert(0, _p)

import numpy as np
from contextlib import ExitStack

import concourse.bass as bass
import concourse.tile as tile
from concourse import bacc, mybir
from concourse.bass_utils import run_bass_kernel_spmd
from concourse.tile_rust import add_dep_helper

F32 = mybir.dt.float32
MM_DT = mybir.dt.float16          # matmul operand dtype
MM_NP = np.float16                # host-side dtype for matmul operands
ALU = mybir.AluOpType
ACTF = mybir.ActivationFunctionType

D = 1024          # model dim
S = 1024          # sequence length
DH = 64           # head dim
JG = 256          # j-columns per head group (4 heads)
KT = 8            # k-tiles of 128 over D
PI = float(np.pi)


def _dep(frm, to, reason):
    """Scheduling-order dependency: `to` must come after `frm`.

    add_dep_helper's arg order is (waiter, prerequisite).
    """
    add_dep_helper(to.ins, frm.ins, sync=False, reason=reason)


def _build_kernel(tc, ins, outs):
    nc = tc.nc
    ctx = ExitStack()

    persist = ctx.enter_context(tc.tile_pool(name="persist", bufs=1))

    # --- persistent tensors (span phases) ---
    q_r = persist.tile([128, 2, S], MM_DT, name="q_r")     # [j%128, pair, s]
    q_in = persist.tile([128, 2, S], MM_DT, name="q_in")   # -(Q'im)
    k_r = persist.tile([128, 2, S], MM_DT, name="k_r")
    k_in = persist.tile([128, 2, S], MM_DT, name="k_in")   # -(K'im)
    k_rn = persist.tile([128, 2, S], MM_DT, name="k_rn")   # -(K're)
    v = persist.tile([128, KT, 3, JG], MM_DT, name="v")    # comps: (-Vi, Vr, Vi)
    attn_re = persist.tile([128, 2, S], MM_DT, name="attn_re")  # [j%128, pair, s]
    attn_im = persist.tile([128, 2, S], MM_DT, name="attn_im")
    ones_col = persist.tile([128, 1], MM_DT, name="ones_col")
    nc.vector.memset(ones_col, 1.0)

    # =================== Phase A: QKV projections + RoPE ===================
    with tc.tile_pool(name="phA", bufs=1) as pa, \
         tc.tile_pool(name="phA_w", bufs=3) as pw, \
         tc.tile_pool(name="phA_tmp", bufs=6) as pt, \
         tc.tile_pool(name="phA_ps", bufs=4, space="PSUM") as ppqk, \
         tc.tile_pool(name="phA_psv", bufs=2, space="PSUM") as ppv:

        # Inputs spread over several DMA queues so the lead-in isn't serial.
        tabs = {}
        for t in ("qc8", "qs8", "kcos", "ksin"):
            tt = pa.tile([128, S], F32, name=f"tab_{t}", tag=f"tab_{t}")
            nc.scalar.dma_start(out=tt, in_=ins[t])
            tabs[t] = tt

        x_re = pa.tile([128, KT, S], MM_DT, name="x_re", tag="x_re")
        x_im = pa.tile([128, KT, S], MM_DT, name="x_im", tag="x_im")
        nc.sync.dma_start(out=x_re, in_=ins["xT_re"].rearrange("(kt p) s -> p kt s", p=128))
        nc.sync.dma_start(out=x_im, in_=ins["xT_im"].rearrange("(kt p) s -> p kt s", p=128))

        wtiles = {}
        for wname, dma_eng in (("wq", nc.gpsimd), ("wk", nc.scalar)):
            w_re = pw.tile([128, KT, 2, 128], MM_DT, name=f"{wname}_re", tag=f"{wname}_re")
            w_im = pw.tile([128, KT, 2, 128], MM_DT, name=f"{wname}_im", tag=f"{wname}_im")
            w_imn = pw.tile([128, KT, 2, 128], MM_DT, name=f"{wname}_imn", tag=f"{wname}_imn")
            # DRAM wT is [D, 256]; [d, pair*128 + j] -> [p, kt, pair, j]
            # (one DMA per pair keeps each access pattern <= 3 dims)
            for wt, sfx in ((w_re, "T_re"), (w_im, "T_im"), (w_imn, "T_imn")):
                src = ins[wname + sfx].rearrange(
                    "(kt p) (pair j) -> p kt pair j", p=128, pair=2)
                for pr in range(2):
                    dma_eng.dma_start(out=wt[:, :, pr, :], in_=src[:, :, pr, :])
            wtiles[wname] = (w_re, w_im, w_imn)
        wv = pw.tile([128, KT, 3, 256], MM_DT, name="wv", tag="wv", bufs=1)
        # comps in free dim: 0=T_imn, 1=T_re, 2=T_im so that
        #   rhs1 = comps[1:3] = [re | im]   (with lhsT = x_re)
        #   rhs2 = comps[0:2] = [imn | re]  (with lhsT = x_im)
        for ci, sfx in ((0, "T_imn"), (1, "T_re"), (2, "T_im")):
            nc.gpsimd.dma_start(
                out=wv[:, :, ci, :],
                in_=ins["wv" + sfx].rearrange("(kt p) j -> p kt j", p=128))

        # ---- Q and K projections: psum[j%128, s] per (pair, s-tile) ----
        for wname, ctab, stab in (("wq", "qc8", "qs8"), ("wk", "kcos", "ksin")):
            w_re, w_im, w_imn = wtiles[wname]
            for pair in range(2):
                for st in range(2):
                    ssl = slice(st * 512, st * 512 + 512)
                    ps_r = ppqk.tile([128, 512], F32, name="ps_r", tag="ps_qk")
                    ps_i = ppqk.tile([128, 512], F32, name="ps_i", tag="ps_qk")
                    for kt in range(KT):
                        lw_re = w_re[:, kt, pair, :]
                        lw_im = w_im[:, kt, pair, :]
                        lw_imn = w_imn[:, kt, pair, :]
                        xr = x_re[:, kt, ssl]
                        xi = x_im[:, kt, ssl]
                        nc.tensor.matmul(ps_r, lhsT=lw_re, rhs=xr,
                                         start=(kt == 0), stop=False)
                        nc.tensor.matmul(ps_i, lhsT=lw_re, rhs=xi,
                                         start=(kt == 0), stop=False)
                        nc.tensor.matmul(ps_r, lhsT=lw_imn, rhs=xi,
                                         start=False, stop=(kt == KT - 1))
                        nc.tensor.matmul(ps_i, lhsT=lw_im, rhs=xr,
                                         start=False, stop=(kt == KT - 1))
                    # RoPE products: p1=Tr*c p2=Ti*s p3=Tr*s p4=Ti*c
                    ct = tabs[ctab][:, ssl]
                    st_t = tabs[stab][:, ssl]
                    p1 = pt.tile([128, 512], F32, name="p1", tag="ropetmp")
                    p2 = pt.tile([128, 512], F32, name="p2", tag="ropetmp")
                    p3 = pt.tile([128, 512], F32, name="p3", tag="ropetmp")
                    p4 = pt.tile([128, 512], F32, name="p4", tag="ropetmp")
                    nc.vector.tensor_mul(p1, ps_r, ct)
                    nc.vector.tensor_mul(p2, ps_i, st_t)
                    nc.vector.tensor_mul(p3, ps_r, st_t)
                    nc.vector.tensor_mul(p4, ps_i, ct)
                    if wname == "wq":
                        nc.vector.tensor_sub(q_r[:, pair, ssl], p1, p2)
                        # q_in = -(p3 + p4)
                        nc.vector.scalar_tensor_tensor(
                            q_in[:, pair, ssl], in0=p3, scalar=-1.0, in1=p4,
                            op0=ALU.mult, op1=ALU.subtract)
                    else:
                        nc.vector.tensor_sub(k_r[:, pair, ssl], p1, p2)
                        nc.vector.tensor_sub(k_rn[:, pair, ssl], p2, p1)
                        nc.vector.scalar_tensor_tensor(
                            k_in[:, pair, ssl], in0=p3, scalar=-1.0, in1=p4,
                            op0=ALU.mult, op1=ALU.subtract)

        # ---- V projection: psum[t%128, 0:256]=Vr-part, [256:512]=Vi-part ----
        for tblk in range(KT):
            ps_v = ppv.tile([128, 512], F32, name="ps_v", tag="ps_v")
            for kt in range(KT):
                lx_re = x_re[:, kt, tblk * 128:(tblk + 1) * 128]
                lx_im = x_im[:, kt, tblk * 128:(tblk + 1) * 128]
                nc.tensor.matmul(ps_v, lhsT=lx_re, rhs=wv[:, kt, 1:3, :],
                                 start=(kt == 0), stop=False)
                nc.tensor.matmul(ps_v, lhsT=lx_im, rhs=wv[:, kt, 0:2, :],
                                 start=False, stop=(kt == KT - 1))
            # copy out (ACT; idle in phase A): v comps (0: -Vi, 1: Vr, 2: Vi)
            nc.scalar.copy(v[:, tblk, 1, :], ps_v[:, 0:256])
            nc.scalar.copy(v[:, tblk, 2, :], ps_v[:, 256:512])
            nc.scalar.activation(v[:, tblk, 0, :], ps_v[:, 256:512],
                                 ACTF.Copy, scale=-1.0)

    # =================== Phase B: attention ===================
    # chunk = (head-pair, s-half); process chunks in groups of 2 sharing one
    # exp-table phase and one sin-table phase (4 ACT table loads total).
    CHUNKS = [(pair, sh) for pair in range(2) for sh in range(2)]
    with tc.tile_pool(name="phB_E", bufs=32) as pE, \
         tc.tile_pool(name="phB_W", bufs=44) as pW, \
         tc.tile_pool(name="phB_WC", bufs=44) as pWC, \
         tc.tile_pool(name="phB_sm", bufs=4) as psm, \
         tc.tile_pool(name="phB_tmp", bufs=8) as pbt, \
         tc.tile_pool(name="phB_ps", bufs=4, space="PSUM") as ppsc, \
         tc.tile_pool(name="phB_psr", bufs=2, space="PSUM") as ppr, \
         tc.tile_pool(name="phB_psa", bufs=2, space="PSUM") as ppa:

        prev_last_sin = None
        for grp in range(2):
            group = CHUNKS[grp * 2:(grp + 1) * 2]
            stash = {}  # (pair, sh) -> (E_tiles, W_tiles, WC_tiles, r_ps)
            exp_insts = []
            # ---- exp sub-phase for both chunks (ACT table: exp) ----
            for pair, sh in group:
                ssl = slice(sh * 512, sh * 512 + 512)
                E_tiles, W_tiles, WC_tiles = {}, {}, {}
                r_ps = ppr.tile([128, 512], F32, name="r_ps", tag="ps_r")
                for tblk in range(KT):
                    tsl = slice(tblk * 128, tblk * 128 + 128)
                    pss = {}
                    for hh in range(2):
                        pss[hh] = (
                            ppsc.tile([128, 512], F32, name="ps_sre", tag="ps_sc"),
                            ppsc.tile([128, 512], F32, name="ps_sip", tag="ps_sc"),
                        )
                    # interleave hh so the two k=64 row-groups overlap on PE
                    for chain in range(4):
                        for hh in range(2):
                            dsl = slice(hh * 64, hh * 64 + 64)
                            ps_re, ps_ip = pss[hh]
                            lkr = k_r[dsl, pair, tsl]
                            lkin = k_in[dsl, pair, tsl]
                            lkrn = k_rn[dsl, pair, tsl]
                            rqr = q_r[dsl, pair, ssl]
                            rqin = q_in[dsl, pair, ssl]
                            if chain == 0:
                                nc.tensor.matmul(ps_re, lhsT=lkr, rhs=rqr,
                                                 start=True, stop=False)
                            elif chain == 1:
                                nc.tensor.matmul(ps_ip, lhsT=lkrn, rhs=rqin,
                                                 start=True, stop=False)
                            elif chain == 2:
                                nc.tensor.matmul(ps_re, lhsT=lkin, rhs=rqin,
                                                 start=False, stop=True)
                            else:
                                nc.tensor.matmul(ps_ip, lhsT=lkin, rhs=rqr,
                                                 start=False, stop=True)
                    for hh in range(2):
                        ps_re, ps_ip = pss[hh]
                        Et = pE.tile([128, 512], MM_DT, name="Et", tag="E")
                        ei = nc.scalar.activation(Et, ps_re, ACTF.Exp)
                        exp_insts.append(ei)
                        if prev_last_sin is not None:
                            _dep(prev_last_sin, ei, "act-table: exp after prev sins")
                        nc.tensor.matmul(r_ps[hh * 64:hh * 64 + 1, :],
                                         lhsT=ones_col, rhs=Et,
                                         start=(tblk == 0), stop=(tblk == KT - 1))
                        Wt = pW.tile([128, 512], MM_DT, name="Wt", tag="W")
                        nc.vector.add_range_wrap(Wt, ps_ip, shift=0.0,
                                                 bound=PI, period=2.0 * PI)
                        WCt = pWC.tile([128, 512], MM_DT, name="WCt", tag="WC")
                        nc.vector.add_range_wrap(WCt, ps_ip, shift=PI / 2.0,
                                                 bound=PI, period=2.0 * PI)
                        E_tiles[(tblk, hh)] = Et
                        W_tiles[(tblk, hh)] = Wt
                        WC_tiles[(tblk, hh)] = WCt
                # denominators -> broadcast -> fast reciprocal (still exp-block)
                Rb = {}
                for hh in range(2):
                    rrow = psm.tile([1, 512], F32, name="rrow", tag="rrow")
                    nc.vector.tensor_copy(rrow, r_ps[hh * 64:hh * 64 + 1, :])
                    rbraw = psm.tile([64, 512], F32, name="rbraw", tag="rbraw")
                    nc.gpsimd.partition_broadcast(rbraw, rrow)
                    rb = psm.tile([64, 512], F32, name="rb", tag="rb")
                    nc.vector.reciprocal_approx_fast(rb, rbraw)
                    Rb[hh] = rb
                stash[(pair, sh)] = (E_tiles, W_tiles, WC_tiles, Rb)

            last_exp = exp_insts[-1]
            # ---- sin sub-phase + AV for both chunks (ACT table: trig) ----
            for pair, sh in group:
                ssl = slice(sh * 512, sh * 512 + 512)
                E_tiles, W_tiles, WC_tiles, Rb = stash[(pair, sh)]
                at_re = ppa.tile([128, 512], F32, name="at_re", tag="ps_at")
                at_im = ppa.tile([128, 512], F32, name="at_im", tag="ps_at")
                for tblk in range(KT):
                    mm_args = {}
                    for hh in range(2):
                        Et = E_tiles[(tblk, hh)]
                        cw = pbt.tile([128, 512], MM_DT, name="cw", tag="sintmp")
                        si1 = nc.scalar.activation(cw, WC_tiles[(tblk, hh)], ACTF.Sin)
                        sw = pbt.tile([128, 512], MM_DT, name="sw", tag="sintmp")
                        si2 = nc.scalar.activation(sw, W_tiles[(tblk, hh)], ACTF.Sin)
                        _dep(last_exp, si1, "act-table: sins after exps")
                        _dep(last_exp, si2, "act-table: sins after exps")
                        prev_last_sin = si2
                        ar = pbt.tile([128, 512], MM_DT, name="ar", tag="avr")
                        nc.gpsimd.tensor_mul(ar, Et, cw)         # exp*cos
                        ai = pbt.tile([128, 512], MM_DT, name="ai", tag="avr")
                        nc.gpsimd.tensor_mul(ai, Et, sw)         # exp*sin
                        mm_args[hh] = (ar, ai)
                    # AV matmuls, hh-interleaved for col-group overlap
                    for chain in range(4):
                        for hh in range(2):
                            ar, ai = mm_args[hh]
                            jsl = slice(pair * 128 + hh * 64, pair * 128 + hh * 64 + 64)
                            psl = slice(hh * 64, hh * 64 + 64)
                            lvin = v[:, tblk, 0, jsl]
                            lvr = v[:, tblk, 1, jsl]
                            lvi = v[:, tblk, 2, jsl]
                            if chain == 0:
                                nc.tensor.matmul(at_re[psl, :], lhsT=lvr, rhs=ar,
                                                 start=(tblk == 0), stop=False)
                            elif chain == 1:
                                nc.tensor.matmul(at_im[psl, :], lhsT=lvi, rhs=ar,
                                                 start=(tblk == 0), stop=False)
                            elif chain == 2:
                                nc.tensor.matmul(at_re[psl, :], lhsT=lvin, rhs=ai,
                                                 start=False, stop=(tblk == KT - 1))
                            else:
                                nc.tensor.matmul(at_im[psl, :], lhsT=lvr, rhs=ai,
                                                 start=False, stop=(tblk == KT - 1))
                # normalize + copy out
                for hh in range(2):
                    psl = slice(hh * 64, hh * 64 + 64)
                    jj = slice(hh * 64, hh * 64 + 64)
                    nc.vector.tensor_mul(
                        attn_re[jj, pair, ssl], at_re[psl, :], Rb[hh])
                    nc.vector.tensor_mul(
                        attn_im[jj, pair, ssl], at_im[psl, :], Rb[hh])

    # =================== Phase C: output projection ===================
    with tc.tile_pool(name="phC", bufs=1) as pc, \
         tc.tile_pool(name="phC_o", bufs=4) as po, \
         tc.tile_pool(name="phC_ps", bufs=4, space="PSUM") as ppc:

        wo = {}
        for sfx in ("T_re", "T_im", "T_imn"):
            wt = pc.tile([128, 2, S], MM_DT, name=f"wo{sfx}", tag=f"wo{sfx}")
            nc.sync.dma_start(
                out=wt, in_=ins["wo" + sfx].rearrange("(kt p) n -> p kt n", p=128))
            wo[sfx] = wt

        for sblk in range(KT):
            bsl = slice(sblk * 128, sblk * 128 + 128)
            for nt in range(2):
                nsl = slice(nt * 512, nt * 512 + 512)
                ps_or = ppc.tile([128, 512], F32, name="ps_or", tag="ps_out")
                ps_oi = ppc.tile([128, 512], F32, name="ps_oi", tag="ps_out")
                for kt in range(2):  # contraction over j (= pair dim)
                    la_r = attn_re[:, kt, bsl]
                    la_i = attn_im[:, kt, bsl]
                    nc.tensor.matmul(ps_or, lhsT=la_r, rhs=wo["T_re"][:, kt, nsl],
                                     start=(kt == 0), stop=False)
                    nc.tensor.matmul(ps_oi, lhsT=la_r, rhs=wo["T_im"][:, kt, nsl],
                                     start=(kt == 0), stop=False)
                    nc.tensor.matmul(ps_or, lhsT=la_i, rhs=wo["T_imn"][:, kt, nsl],
                                     start=False, stop=(kt == 1))
                    nc.tensor.matmul(ps_oi, lhsT=la_i, rhs=wo["T_re"][:, kt, nsl],
                                     start=False, stop=(kt == 1))
                o_r = po.tile([128, 512], F32, name="o_r", tag="otmp")
                o_i = po.tile([128, 512], F32, name="o_i", tag="otmp")
                nc.vector.tensor_copy(o_r, ps_or)
                nc.vector.tensor_copy(o_i, ps_oi)
                nc.sync.dma_start(out=outs["out_re"][bsl, nsl], in_=o_r)
                nc.sync.dma_start(out=outs["out_im"][bsl, nsl], in_=o_i)

    ctx.close()


_IN_SPECS = (
    [("xT_re", [D, S], MM_DT), ("xT_im", [D, S], MM_DT)]
    + [(w + sfx, [D, JG], MM_DT) for w in ("wq", "wk", "wv")
       for sfx in ("T_re", "T_im", "T_imn")]
    + [("wo" + sfx, [JG, D], MM_DT) for sfx in ("T_re", "T_im", "T_imn")]
    + [(t, [128, S], F32) for t in ("qc8", "qs8", "kcos", "ksin")]
)


def build_program():
    nc = bacc.Bacc("TRN2", target_bir_lowering=False, debug=False,
                   enable_asserts=False, num_devices=8)
    ins = {name: nc.dram_tensor(name, shape, dt, kind="ExternalInput").ap()
           for name, shape, dt in _IN_SPECS}
    outs = {name: nc.dram_tensor(name, [S, D], F32, kind="ExternalOutput").ap()
            for name in ("out_re", "out_im")}
    with tile.TileContext(nc) as tc:
        _build_kernel(tc, ins, outs)
    nc.compile()
    return nc


def _make_tables():
    inv_freq = 1.0 / (10000.0 ** (np.arange(DH, dtype=np.float64) / DH))
    ang = np.arange(S, dtype=np.float64)[:, None] * inv_freq[None, :]  # [S, DH]
    angT = ang.T  # [DH, S]
    ang128 = np.concatenate([angT, angT], axis=0)  # [128, S]
    c = np.cos(ang128)
    s = np.sin(ang128)
    return {
        "qc8": (c * 0.125).astype(np.float32),
        "qs8": (s * 0.125).astype(np.float32),
        "kcos": c.astype(np.float32),
        "ksin": s.astype(np.float32),
    }


def _core_inputs(inputs, c, tables):
    b, g = divmod(c, 4)
    rows = slice(g * JG, (g + 1) * JG)

    def f(a):
        return np.ascontiguousarray(np.asarray(a, dtype=np.float32)).astype(MM_NP)

    m = {
        "xT_re": f(np.asarray(inputs["x_re"])[b].T),
        "xT_im": f(np.asarray(inputs["x_im"])[b].T),
        "woT_re": f(np.asarray(inputs["wo_re"])[:, rows].T),
        "woT_im": f(np.asarray(inputs["wo_im"])[:, rows].T),
        "woT_imn": f(-np.asarray(inputs["wo_im"])[:, rows].T),
    }
    for w in ("wq", "wk", "wv"):
        wre = np.asarray(inputs[w + "_re"])[rows]
        wim = np.asarray(inputs[w + "_im"])[rows]
        m[w + "T_re"] = f(wre.T)
        m[w + "T_im"] = f(wim.T)
        m[w + "T_imn"] = f(-wim.T)
    m.update(tables)
    return m


_PROGRAM = None


def _get_program():
    global _PROGRAM
    if _PROGRAM is None:
        _PROGRAM = build_program()
    return _PROGRAM


def run(inputs, trace=False, **kwargs):
    nc = _get_program()
    tables = _make_tables()
    in_maps = [_core_inputs(inputs, c, tables) for c in range(8)]
    res = run_bass_kernel_spmd(nc, in_maps, list(range(8)), trace=trace, **kwargs)
    B = 2
    out = np.zeros((B, S, D, 2), np.float32)
    for c, r in enumerate(res.results):
        b = c // 4
        out[b, :, :, 0] += r["out_re"]
        out[b, :, :, 1] += r["out_im"]
    return out, res


def kernel(**inputs):
    out, _ = run(inputs)
    return out


if __name__ == "__main__":
    nc = build_program()
    print("program built + compiled OK")


# revision 14
# speedup vs baseline: 1.3796x; 1.3796x over previous
"""Trainium2 Bass kernel for CartesianDecomposedAttention (complex-valued attention).

Reference math (complex):
  Q = (x @ wq.T) * rotor ; K = (x @ wk.T) * rotor ; V = x @ wv.T
  scores = Q conj(K)^T / sqrt(Dh)
  attn_w = softmax(scores.re) * exp(i * scores.im)
  out    = (attn_w @ V) @ wo.T        -> stack([re, im], -1)

Sharding over 8 cores: core c -> batch b=c//4, head-group g=c%4 (4 heads, 256
model dims per group). Each core computes a partial output [S, D] (re+im);
the host sums the 4 group partials per batch (no on-device collectives).

Device decomposition (matmul operands fp16, PSUM accumulation fp32,
everything transposed so no on-chip transposes are needed):
  - Host passes x^T, w^T slices (fp16), and negated copies where a
    PSUM-accumulation subtraction is needed (PSUM can only add).
  - RoPE via host-built cos/sin tables [128, S] (fp32); the 1/sqrt(Dh) score
    scale is folded into the Q-side tables. Stored components are chosen so
    both scoresT.re and scoresT.im are pure PSUM additions:
      Q: (Qr, Qin=-Qi)   K: (Kr, Kin=-Ki, Krn=-Kr)   V: (Vin=-Vi, Vr, Vi)
      Sr  = Kr.T Qr + Kin.T Qin          (= scores.re)
      Sip = Krn.T Qin + Kin.T Qr         (= +scores.im)
      attn_re = Vr.T ar + Vin.T ai ; attn_im = Vi.T ar + Vr.T ai
  - softmax without max-subtraction (scores in [-8, 8]); the denominator is a
    ones-vector matmul over exp tiles, applied *after* the AV matmul
    (1/r via reciprocal_approx_fast on a gpsimd partition_broadcast tile).
  - HW Sin is valid only on [-pi, pi]: two add_range_wraps straight from the
    scores.im PSUM tile (shift 0 -> sin arg, shift pi/2 -> cos arg; inputs
    stay within one 2pi period of [-pi, pi]).
  - ACT exp and sin live in different table sets (~2.7us per switch): chunks
    are processed in groups of two with all exps batched before all sins,
    enforced with explicit scheduling deps.
"""

import sys

for _p in ("/opt/trn_rl_repo",):
    if _p not in sys.path:
        sys.path.insert(0, _p)

import numpy as np
from contextlib import ExitStack

import concourse.bass as bass
import concourse.tile as tile
from concourse import bacc, mybir
from concourse.bass_utils import run_bass_kernel_spmd
from concourse.tile_rust import add_dep_helper

F32 = mybir.dt.float32
MM_DT = mybir.dt.float16          # matmul operand dtype
MM_NP = np.float16                # host-side dtype for matmul operands
ALU = mybir.AluOpType
ACTF = mybir.ActivationFunctionType

D = 1024          # model dim
S = 1024          # sequence length
DH = 64           # head dim
JG = 256          # j-columns per head group (4 heads)
KT = 8            # k-tiles of 128 over D
PI = float(np.pi)


def _dep(frm, to, reason):
    """Scheduling-order dependency: `to` must come after `frm`.

    add_dep_helper's arg order is (waiter, prerequisite).
    """
    add_dep_helper(to.ins, frm.ins, sync=False, reason=reason)


def _build_kernel(tc, ins, outs):
    nc = tc.nc
    ctx = ExitStack()

    persist = ctx.enter_context(tc.tile_pool(name="persist", bufs=1))

    # --- persistent tensors (span phases) ---
    q_r = persist.tile([128, 2, S], MM_DT, name="q_r")     # [j%128, pair, s]
    q_in = persist.tile([128, 2, S], MM_DT, name="q_in")   # -(Q'im)
    k_r = persist.tile([128, 2, S], MM_DT, name="k_r")
    k_in = persist.tile([128, 2, S], MM_DT, name="k_in")   # -(K'im)
    k_rn = persist.tile([128, 2, S], MM_DT, name="k_rn")   # -(K're)
    v = persist.tile([128, KT, 3, JG], MM_DT, name="v")    # comps: (-Vi, Vr, Vi)
    attn_re = persist.tile([128, 2, S], MM_DT, name="attn_re")  # [j%128, pair, s]
    attn_im = persist.tile([128, 2, S], MM_DT, name="attn_im")
    ones_col = persist.tile([128, 1], MM_DT, name="ones_col")
    nc.vector.memset(ones_col, 1.0)

    # =================== Phase A: QKV projections + RoPE ===================
    with tc.tile_pool(name="phA", bufs=1) as pa, \
         tc.tile_pool(name="phA_w", bufs=3) as pw, \
         tc.tile_pool(name="phA_tmp", bufs=6) as pt, \
         tc.tile_pool(name="phA_ps", bufs=4, space="PSUM") as ppqk, \
         tc.tile_pool(name="phA_psv", bufs=2, space="PSUM") as ppv:

        # Inputs spread over several DMA queues so the lead-in isn't serial.
        tabs = {}
        for t in ("qc8", "qs8", "kcos", "ksin"):
            tt = pa.tile([128, S], F32, name=f"tab_{t}", tag=f"tab_{t}")
            nc.sync.dma_start(out=tt, in_=ins[t])
            tabs[t] = tt

        x_re = pa.tile([128, KT, S], MM_DT, name="x_re", tag="x_re")
        x_im = pa.tile([128, KT, S], MM_DT, name="x_im", tag="x_im")
        nc.sync.dma_start(out=x_re, in_=ins["xT_re"].rearrange("(kt p) s -> p kt s", p=128))
        nc.sync.dma_start(out=x_im, in_=ins["xT_im"].rearrange("(kt p) s -> p kt s", p=128))

        wtiles = {}
        for wname, dma_eng in (("wq", nc.gpsimd), ("wk", nc.gpsimd)):
            w_re = pw.tile([128, KT, 2, 128], MM_DT, name=f"{wname}_re", tag=f"{wname}_re")
            w_im = pw.tile([128, KT, 2, 128], MM_DT, name=f"{wname}_im", tag=f"{wname}_im")
            w_imn = pw.tile([128, KT, 2, 128], MM_DT, name=f"{wname}_imn", tag=f"{wname}_imn")
            # DRAM wT is [D, 256]; [d, pair*128 + j] -> [p, kt, pair, j]
            # (one DMA per pair keeps each access pattern <= 3 dims)
            for wt, sfx in ((w_re, "T_re"), (w_im, "T_im"), (w_imn, "T_imn")):
                src = ins[wname + sfx].rearrange(
                    "(kt p) (pair j) -> p kt pair j", p=128, pair=2)
                for pr in range(2):
                    dma_eng.dma_start(out=wt[:, :, pr, :], in_=src[:, :, pr, :])
            wtiles[wname] = (w_re, w_im, w_imn)
        wv = pw.tile([128, KT, 3, 256], MM_DT, name="wv", tag="wv", bufs=1)
        # comps in free dim: 0=T_imn, 1=T_re, 2=T_im so that
        #   rhs1 = comps[1:3] = [re | im]   (with lhsT = x_re)
        #   rhs2 = comps[0:2] = [imn | re]  (with lhsT = x_im)
        for ci, sfx in ((0, "T_imn"), (1, "T_re"), (2, "T_im")):
            nc.gpsimd.dma_start(
                out=wv[:, :, ci, :],
                in_=ins["wv" + sfx].rearrange("(kt p) j -> p kt j", p=128))

        # ---- Q and K projections: psum[j%128, s] per (pair, s-tile) ----
        for wname, ctab, stab in (("wq", "qc8", "qs8"), ("wk", "kcos", "ksin")):
            w_re, w_im, w_imn = wtiles[wname]
            for pair in range(2):
                for st in range(2):
                    ssl = slice(st * 512, st * 512 + 512)
                    ps_r = ppqk.tile([128, 512], F32, name="ps_r", tag="ps_qk")
                    ps_i = ppqk.tile([128, 512], F32, name="ps_i", tag="ps_qk")
                    for kt in range(KT):
                        lw_re = w_re[:, kt, pair, :]
                        lw_im = w_im[:, kt, pair, :]
                        lw_imn = w_imn[:, kt, pair, :]
                        xr = x_re[:, kt, ssl]
                        xi = x_im[:, kt, ssl]
                        nc.tensor.matmul(ps_r, lhsT=lw_re, rhs=xr,
                                         start=(kt == 0), stop=False)
                        nc.tensor.matmul(ps_i, lhsT=lw_re, rhs=xi,
                                         start=(kt == 0), stop=False)
                        nc.tensor.matmul(ps_r, lhsT=lw_imn, rhs=xi,
                                         start=False, stop=(kt == KT - 1))
                        nc.tensor.matmul(ps_i, lhsT=lw_im, rhs=xr,
                                         start=False, stop=(kt == KT - 1))
                    # RoPE products: p1=Tr*c p2=Ti*s p3=Tr*s p4=Ti*c
                    ct = tabs[ctab][:, ssl]
                    st_t = tabs[stab][:, ssl]
                    p1 = pt.tile([128, 512], F32, name="p1", tag="ropetmp")
                    p2 = pt.tile([128, 512], F32, name="p2", tag="ropetmp")
                    p3 = pt.tile([128, 512], F32, name="p3", tag="ropetmp")
                    p4 = pt.tile([128, 512], F32, name="p4", tag="ropetmp")
                    nc.vector.tensor_mul(p1, ps_r, ct)
                    nc.vector.tensor_mul(p2, ps_i, st_t)
                    nc.vector.tensor_mul(p3, ps_r, st_t)
                    nc.vector.tensor_mul(p4, ps_i, ct)
                    if wname == "wq":
                        nc.vector.tensor_sub(q_r[:, pair, ssl], p1, p2)
                        # q_in = -(p3 + p4)
                        nc.vector.scalar_tensor_tensor(
                            q_in[:, pair, ssl], in0=p3, scalar=-1.0, in1=p4,
                            op0=ALU.mult, op1=ALU.subtract)
                    else:
                        nc.vector.tensor_sub(k_r[:, pair, ssl], p1, p2)
                        nc.vector.tensor_sub(k_rn[:, pair, ssl], p2, p1)
                        nc.vector.scalar_tensor_tensor(
                            k_in[:, pair, ssl], in0=p3, scalar=-1.0, in1=p4,
                            op0=ALU.mult, op1=ALU.subtract)

        # ---- V projection: psum[t%128, 0:256]=Vr-part, [256:512]=Vi-part ----
        for tblk in range(KT):
            ps_v = ppv.tile([128, 512], F32, name="ps_v", tag="ps_v")
            for kt in range(KT):
                lx_re = x_re[:, kt, tblk * 128:(tblk + 1) * 128]
                lx_im = x_im[:, kt, tblk * 128:(tblk + 1) * 128]
                nc.tensor.matmul(ps_v, lhsT=lx_re, rhs=wv[:, kt, 1:3, :],
                                 start=(kt == 0), stop=False)
                nc.tensor.matmul(ps_v, lhsT=lx_im, rhs=wv[:, kt, 0:2, :],
                                 start=False, stop=(kt == KT - 1))
            # copy out (ACT; idle in phase A): v comps (0: -Vi, 1: Vr, 2: Vi)
            nc.scalar.copy(v[:, tblk, 1, :], ps_v[:, 0:256])
            nc.scalar.copy(v[:, tblk, 2, :], ps_v[:, 256:512])
            nc.scalar.activation(v[:, tblk, 0, :], ps_v[:, 256:512],
                                 ACTF.Copy, scale=-1.0)

    # =================== Phase B: attention ===================
    # chunk = (head-pair, s-half); process chunks in groups of 2 sharing one
    # exp-table phase and one sin-table phase (4 ACT table loads total).
    CHUNKS = [(pair, sh) for pair in range(2) for sh in range(2)]
    with tc.tile_pool(name="phB_E", bufs=32) as pE, \
         tc.tile_pool(name="phB_W", bufs=44) as pW, \
         tc.tile_pool(name="phB_WC", bufs=44) as pWC, \
         tc.tile_pool(name="phB_sm", bufs=4) as psm, \
         tc.tile_pool(name="phB_tmp", bufs=8) as pbt, \
         tc.tile_pool(name="phB_ps", bufs=4, space="PSUM") as ppsc, \
         tc.tile_pool(name="phB_psr", bufs=2, space="PSUM") as ppr, \
         tc.tile_pool(name="phB_psa", bufs=2, space="PSUM") as ppa:

        prev_last_sin = None
        for grp in range(2):
            group = CHUNKS[grp * 2:(grp + 1) * 2]
            stash = {}  # (pair, sh) -> (E_tiles, W_tiles, WC_tiles, r_ps)
            exp_insts = []
            # ---- exp sub-phase for both chunks (ACT table: exp) ----
            for pair, sh in group:
                ssl = slice(sh * 512, sh * 512 + 512)
                E_tiles, W_tiles, WC_tiles = {}, {}, {}
                r_ps = ppr.tile([128, 512], F32, name="r_ps", tag="ps_r")
                for tblk in range(KT):
                    tsl = slice(tblk * 128, tblk * 128 + 128)
                    pss = {}
                    for hh in range(2):
                        pss[hh] = (
                            ppsc.tile([128, 512], F32, name="ps_sre", tag="ps_sc"),
                            ppsc.tile([128, 512], F32, name="ps_sip", tag="ps_sc"),
                        )
                    # interleave hh so the two k=64 row-groups overlap on PE
                    for chain in range(4):
                        for hh in range(2):
                            dsl = slice(hh * 64, hh * 64 + 64)
                            ps_re, ps_ip = pss[hh]
                            lkr = k_r[dsl, pair, tsl]
                            lkin = k_in[dsl, pair, tsl]
                            lkrn = k_rn[dsl, pair, tsl]
                            rqr = q_r[dsl, pair, ssl]
                            rqin = q_in[dsl, pair, ssl]
                            if chain == 0:
                                nc.tensor.matmul(ps_re, lhsT=lkr, rhs=rqr,
                                                 start=True, stop=False)
                            elif chain == 1:
                                nc.tensor.matmul(ps_ip, lhsT=lkrn, rhs=rqin,
                                                 start=True, stop=False)
                            elif chain == 2:
                                nc.tensor.matmul(ps_re, lhsT=lkin, rhs=rqin,
                                                 start=False, stop=True)
                            else:
                                nc.tensor.matmul(ps_ip, lhsT=lkin, rhs=rqr,
                                                 start=False, stop=True)
                    for hh in range(2):
                        ps_re, ps_ip = pss[hh]
                        Et = pE.tile([128, 512], MM_DT, name="Et", tag="E")
                        ei = nc.scalar.activation(Et, ps_re, ACTF.Exp)
                        exp_insts.append(ei)
                        if prev_last_sin is not None:
                            _dep(prev_last_sin, ei, "act-table: exp after prev sins")
                        nc.tensor.matmul(r_ps[hh * 64:hh * 64 + 1, :],
                                         lhsT=ones_col, rhs=Et,
                                         start=(tblk == 0), stop=(tblk == KT - 1))
                        Wt = pW.tile([128, 512], MM_DT, name="Wt", tag="W")
                        nc.vector.add_range_wrap(Wt, ps_ip, shift=0.0,
                                                 bound=PI, period=2.0 * PI)
                        WCt = pWC.tile([128, 512], MM_DT, name="WCt", tag="WC")
                        nc.vector.add_range_wrap(WCt, ps_ip, shift=PI / 2.0,
                                                 bound=PI, period=2.0 * PI)
                        E_tiles[(tblk, hh)] = Et
                        W_tiles[(tblk, hh)] = Wt
                        WC_tiles[(tblk, hh)] = WCt
                # denominators -> broadcast -> fast reciprocal (still exp-block)
                Rb = {}
                for hh in range(2):
                    rrow = psm.tile([1, 512], F32, name="rrow", tag="rrow")
                    nc.vector.tensor_copy(rrow, r_ps[hh * 64:hh * 64 + 1, :])
                    rbraw = psm.tile([64, 512], F32, name="rbraw", tag="rbraw")
                    nc.gpsimd.partition_broadcast(rbraw, rrow)
                    rb = psm.tile([64, 512], F32, name="rb", tag="rb")
                    nc.vector.reciprocal_approx_fast(rb, rbraw)
                    Rb[hh] = rb
                stash[(pair, sh)] = (E_tiles, W_tiles, WC_tiles, Rb)

            last_exp = exp_insts[-1]
            # ---- sin sub-phase + AV for both chunks (ACT table: trig) ----
            for pair, sh in group:
                ssl = slice(sh * 512, sh * 512 + 512)
                E_tiles, W_tiles, WC_tiles, Rb = stash[(pair, sh)]
                at_re = ppa.tile([128, 512], F32, name="at_re", tag="ps_at")
                at_im = ppa.tile([128, 512], F32, name="at_im", tag="ps_at")
                for tblk in range(KT):
                    mm_args = {}
                    for hh in range(2):
                        Et = E_tiles[(tblk, hh)]
                        cw = pbt.tile([128, 512], MM_DT, name="cw", tag="sintmp")
                        si1 = nc.scalar.activation(cw, WC_tiles[(tblk, hh)], ACTF.Sin)
                        sw = pbt.tile([128, 512], MM_DT, name="sw", tag="sintmp")
                        si2 = nc.scalar.activation(sw, W_tiles[(tblk, hh)], ACTF.Sin)
                        _dep(last_exp, si1, "act-table: sins after exps")
                        _dep(last_exp, si2, "act-table: sins after exps")
                        prev_last_sin = si2
                        ar = pbt.tile([128, 512], MM_DT, name="ar", tag="avr")
                        nc.vector.tensor_mul(ar, Et, cw)         # exp*cos
                        ai = pbt.tile([128, 512], MM_DT, name="ai", tag="avr")
                        nc.vector.tensor_mul(ai, Et, sw)         # exp*sin
                        mm_args[hh] = (ar, ai)
                    # AV matmuls, hh-interleaved for col-group overlap
                    for chain in range(4):
                        for hh in range(2):
                            ar, ai = mm_args[hh]
                            jsl = slice(pair * 128 + hh * 64, pair * 128 + hh * 64 + 64)
                            psl = slice(hh * 64, hh * 64 + 64)
                            lvin = v[:, tblk, 0, jsl]
                            lvr = v[:, tblk, 1, jsl]
                            lvi = v[:, tblk, 2, jsl]
                            if chain == 0:
                                nc.tensor.matmul(at_re[psl, :], lhsT=lvr, rhs=ar,
                                                 start=(tblk == 0), stop=False)
                            elif chain == 1:
                                nc.tensor.matmul(at_im[psl, :], lhsT=lvi, rhs=ar,
                                                 start=(tblk == 0), stop=False)
                            elif chain == 2:
                                nc.tensor.matmul(at_re[psl, :], lhsT=lvin, rhs=ai,
                                                 start=False, stop=(tblk == KT - 1))
                            else:
                                nc.tensor.matmul(at_im[psl, :], lhsT=lvr, rhs=ai,
                                                 start=False, stop=(tblk == KT - 1))
                # normalize + copy out
                for hh in range(2):
                    psl = slice(hh * 64, hh * 64 + 64)
                    jj = slice(hh * 64, hh * 64 + 64)
                    nc.vector.tensor_mul(
                        attn_re[jj, pair, ssl], at_re[psl, :], Rb[hh])
                    nc.vector.tensor_mul(
                        attn_im[jj, pair, ssl], at_im[psl, :], Rb[hh])

    # =================== Phase C: output projection ===================
    with tc.tile_pool(name="phC", bufs=1) as pc, \
         tc.tile_pool(name="phC_o", bufs=4) as po, \
         tc.tile_pool(name="phC_ps", bufs=4, space="PSUM") as ppc:

        wo = {}
        for sfx in ("T_re", "T_im", "T_imn"):
            wt = pc.tile([128, 2, S], MM_DT, name=f"wo{sfx}", tag=f"wo{sfx}")
            nc.sync.dma_start(
                out=wt, in_=ins["wo" + sfx].rearrange("(kt p) n -> p kt n", p=128))
            wo[sfx] = wt

        for sblk in range(KT):
            bsl = slice(sblk * 128, sblk * 128 + 128)
            for nt in range(2):
                nsl = slice(nt * 512, nt * 512 + 512)
                ps_or = ppc.tile([128, 512], F32, name="ps_or", tag="ps_out")
                ps_oi = ppc.tile([128, 512], F32, name="ps_oi", tag="ps_out")
                for kt in range(2):  # contraction over j (= pair dim)
                    la_r = attn_re[:, kt, bsl]
                    la_i = attn_im[:, kt, bsl]
                    nc.tensor.matmul(ps_or, lhsT=la_r, rhs=wo["T_re"][:, kt, nsl],
                                     start=(kt == 0), stop=False)
                    nc.tensor.matmul(ps_oi, lhsT=la_r, rhs=wo["T_im"][:, kt, nsl],
                                     start=(kt == 0), stop=False)
                    nc.tensor.matmul(ps_or, lhsT=la_i, rhs=wo["T_imn"][:, kt, nsl],
                                     start=False, stop=(kt == 1))
                    nc.tensor.matmul(ps_oi, lhsT=la_i, rhs=wo["T_re"][:, kt, nsl],
                                     start=False, stop=(kt == 1))
                o_r = po.tile([128, 512], F32, name="o_r", tag="otmp")
                o_i = po.tile([128, 512], F32, name="o_i", tag="otmp")
                nc.vector.tensor_copy(o_r, ps_or)
                nc.vector.tensor_copy(o_i, ps_oi)
                nc.sync.dma_start(out=outs["out_re"][bsl, nsl], in_=o_r)
                nc.sync.dma_start(out=outs["out_im"][bsl, nsl], in_=o_i)

    ctx.close()


_IN_SPECS = (
    [("xT_re", [D, S], MM_DT), ("xT_im", [D, S], MM_DT)]
    + [(w + sfx, [D, JG], MM_DT) for w in ("wq", "wk", "wv")
       for sfx in ("T_re", "T_im", "T_imn")]
    + [("wo" + sfx, [JG, D], MM_DT) for sfx in ("T_re", "T_im", "T_imn")]
    + [(t, [128, S], F32) for t in ("qc8", "qs8", "kcos", "ksin")]
)


def build_program():
    nc = bacc.Bacc("TRN2", target_bir_lowering=False, debug=False,
                   enable_asserts=False, num_devices=8)
    ins = {name: nc.dram_tensor(name, shape, dt, kind="ExternalInput").ap()
           for name, shape, dt in _IN_SPECS}
    outs = {name: nc.dram_tensor(name, [S, D], F32, kind="ExternalOutput").ap()
            for name in ("out_re", "out_im")}
    with tile.TileContext(nc) as tc:
        _build_kernel(tc, ins, outs)
    nc.compile()
    return nc


def _make_tables():
    inv_freq = 1.0 / (10000.0 ** (np.arange(DH, dtype=np.float64) / DH))
    ang = np.arange(S, dtype=np.float64)[:, None] * inv_freq[None, :]  # [S, DH]
    angT = ang.T  # [DH, S]
    ang128 = np.concatenate([angT, angT], axis=0)  # [128, S]
    c = np.cos(ang128)
    s = np.sin(ang128)
    return {
        "qc8": (c * 0.125).astype(np.float32),
        "qs8": (s * 0.125).astype(np.float32),
        "kcos": c.astype(np.float32),
        "ksin": s.astype(np.float32),
    }


def _core_inputs(inputs, c, tables):
    b, g = divmod(c, 4)
    rows = slice(g * JG, (g + 1) * JG)

    def f(a):
        return np.ascontiguousarray(np.asarray(a, dtype=np.float32)).astype(MM_NP)

    m = {
        "xT_re": f(np.asarray(inputs["x_re"])[b].T),
        "xT_im": f(np.asarray(inputs["x_im"])[b].T),
        "woT_re": f(np.asarray(inputs["wo_re"])[:, rows].T),
        "woT_im": f(np.asarray(inputs["wo_im"])[:, rows].T),
        "woT_imn": f(-np.asarray(inputs["wo_im"])[:, rows].T),
    }
    for w in ("wq", "wk", "wv"):
        wre = np.asarray(inputs[w + "_re"])[rows]
        wim = np.asarray(inputs[w + "_im"])[rows]
        m[w + "T_re"] = f(wre.T)
        m[w + "T_im"] = f(wim.T)
        m[w + "T_imn"] = f(-wim.T)
    m.update(tables)
    return m


_PROGRAM = None


def _get_program():
    global _PROGRAM
    if _PROGRAM is None:
        _PROGRAM = build_program()
    return _PROGRAM


def run(inputs, trace=False, **kwargs):
    nc = _get_program()
    tables = _make_tables()
    in_maps = [_core_inputs(inputs, c, tables) for c in range(8)]
    res = run_bass_kernel_spmd(nc, in_maps, list(range(8)), trace=trace, **kwargs)
    B = 2
    out = np.zeros((B, S, D, 2), np.float32)
    for c, r in enumerate(res.results):
        b = c // 4
        out[b, :, :, 0] += r["out_re"]
        out[b, :, :, 1] += r["out_im"]
    return out, res


def kernel(**inputs):
    out, _ = run(inputs)
    return out


if __name__ == "__main__":
    nc = build_program()
    print("program built + compiled OK")


# revision 17
# speedup vs baseline: 1.6345x; 1.1847x over previous
"""Trainium2 Bass kernel for CartesianDecomposedAttention (complex-valued attention).

Reference math (complex):
  Q = (x @ wq.T) * rotor ; K = (x @ wk.T) * rotor ; V = x @ wv.T
  scores = Q conj(K)^T / sqrt(Dh)
  attn_w = softmax(scores.re) * exp(i * scores.im)
  out    = (attn_w @ V) @ wo.T        -> stack([re, im], -1)

Sharding over 8 cores: core c -> batch b=c//4, head-group g=c%4 (4 heads, 256
model dims per group). Each core computes a partial output [S, D] (re+im);
the host sums the 4 group partials per batch (no on-device collectives).

Device decomposition (matmul operands fp16, PSUM accumulation fp32,
everything transposed so no on-chip transposes are needed):
  - Host passes x^T, w^T slices (fp16), and negated copies where a
    PSUM-accumulation subtraction is needed (PSUM can only add).
  - RoPE via host-built cos/sin tables [128, S]; the 1/sqrt(Dh) score
    scale is folded into the Q-side tables. Stored components are chosen so
    both scoresT.re and scoresT.im are pure PSUM additions:
      Q: (Qr, Qin=-Qi)   K: (Kr, Kin=-Ki, Krn=-Kr)   V: (Vin=-Vi, Vr, Vi)
      Sr  = Kr.T Qr + Kin.T Qin          (= scores.re)
      Sip = Krn.T Qin + Kin.T Qr         (= +scores.im)
      attn_re = Vr.T ar + Vin.T ai ; attn_im = Vi.T ar + Vr.T ai
  - softmax without max-subtraction (scores in [-8, 8]); the denominator is a
    ones-vector matmul over exp tiles, applied *after* the AV matmul
    (1/r via reciprocal_approx_fast on a gpsimd partition_broadcast tile).
  - HW Sin is valid only on [-pi, pi]: two add_range_wraps straight from the
    scores.im PSUM tile (shift 0 -> sin arg, shift pi/2 -> cos arg).
  - ACT exp and sin live in different table sets (~2.7us per switch): the two
    chunks of each head-pair batch all exps before all sins (4 loads total),
    enforced with explicit scheduling deps.
  - Pipelined emission: Q/K(pair0) -> exp(pair0 chunks) overlaps the rest of
    the QKV projections; per-head tiles are packed side by side in
    [128, 1024] SBUF tiles so sins/muls run at N=1024.
"""

import sys

for _p in ("/opt/trn_rl_repo",):
    if _p not in sys.path:
        sys.path.insert(0, _p)

import numpy as np
from contextlib import ExitStack

import concourse.bass as bass
import concourse.tile as tile
from concourse import bacc, mybir
from concourse.bass_utils import run_bass_kernel_spmd
from concourse.tile_rust import add_dep_helper

F32 = mybir.dt.float32
MM_DT = mybir.dt.float16          # matmul operand dtype
MM_NP = np.float16                # host-side dtype for matmul operands
TAB_DT = mybir.dt.float16         # rope table dtype
ALU = mybir.AluOpType
ACTF = mybir.ActivationFunctionType

D = 1024          # model dim
S = 1024          # sequence length
DH = 64           # head dim
JG = 256          # j-columns per head group (4 heads)
KT = 8            # k-tiles of 128 over D
PI = float(np.pi)


def _dep(frm, to, reason):
    """Scheduling-order dependency: `to` must come after `frm`.

    add_dep_helper's arg order is (waiter, prerequisite).
    """
    add_dep_helper(to.ins, frm.ins, sync=False, reason=reason)


def _build_kernel(tc, ins, outs):
    nc = tc.nc
    ctx = ExitStack()

    persist = ctx.enter_context(tc.tile_pool(name="persist", bufs=1))
    # shared PSUM pools: "mm" serves QKV projections, scores and the output
    # projection; "at" the AV accumulators; "r" the softmax denominators.
    pmm = ctx.enter_context(tc.tile_pool(name="ps_mm", bufs=4, space="PSUM"))
    pat = ctx.enter_context(tc.tile_pool(name="ps_at", bufs=2, space="PSUM"))
    ppr = ctx.enter_context(tc.tile_pool(name="ps_r", bufs=2, space="PSUM"))

    # --- persistent tensors (span phases) ---
    q_r = persist.tile([128, 2, S], MM_DT, name="q_r")     # [j%128, pair, s]
    q_in = persist.tile([128, 2, S], MM_DT, name="q_in")   # -(Q'im)
    k_r = persist.tile([128, 2, S], MM_DT, name="k_r")
    k_in = persist.tile([128, 2, S], MM_DT, name="k_in")   # -(K'im)
    k_rn = persist.tile([128, 2, S], MM_DT, name="k_rn")   # -(K're)
    v = persist.tile([128, KT, 3, JG], MM_DT, name="v")    # comps: (-Vi, Vr, Vi)
    attn_re = persist.tile([128, 2, S], MM_DT, name="attn_re")  # [j%128, pair, s]
    attn_im = persist.tile([128, 2, S], MM_DT, name="attn_im")
    ones_col = persist.tile([128, 1], MM_DT, name="ones_col")
    nc.vector.memset(ones_col, 1.0)

    # phase-B pools (outer so they survive until the end of attention)
    pB = ExitStack()
    pE = pB.enter_context(tc.tile_pool(name="phB_E", bufs=8))
    pW = pB.enter_context(tc.tile_pool(name="phB_W", bufs=8))
    pWC = pB.enter_context(tc.tile_pool(name="phB_WC", bufs=8))
    psm = pB.enter_context(tc.tile_pool(name="phB_sm", bufs=4))
    pbt = pB.enter_context(tc.tile_pool(name="phB_tmp", bufs=4))

    # phase-A pools (inner; released after the V projection to make room)
    phA = ExitStack()
    pa = phA.enter_context(tc.tile_pool(name="phA", bufs=1))
    pw = phA.enter_context(tc.tile_pool(name="phA_w", bufs=2))
    pt = phA.enter_context(tc.tile_pool(name="phA_tmp", bufs=6))

    tabs = {}
    for t in ("qc8", "qs8", "kcos", "ksin"):
        tt = pa.tile([128, S], TAB_DT, name=f"tab_{t}", tag=f"tab_{t}")
        nc.sync.dma_start(out=tt, in_=ins[t])
        tabs[t] = tt

    x_re = pa.tile([128, KT, S], MM_DT, name="x_re", tag="x_re")
    x_im = pa.tile([128, KT, S], MM_DT, name="x_im", tag="x_im")
    nc.sync.dma_start(out=x_re, in_=ins["xT_re"].rearrange("(kt p) s -> p kt s", p=128))
    nc.sync.dma_start(out=x_im, in_=ins["xT_im"].rearrange("(kt p) s -> p kt s", p=128))

    wv = pw.tile([128, KT, 3, 256], MM_DT, name="wv", tag="wv", bufs=1)
    # comps in free dim: 0=T_imn, 1=T_re, 2=T_im so that
    #   rhs1 = comps[1:3] = [re | im]   (with lhsT = x_re)
    #   rhs2 = comps[0:2] = [imn | re]  (with lhsT = x_im)
    for ci, sfx in ((0, "T_imn"), (1, "T_re"), (2, "T_im")):
        nc.gpsimd.dma_start(
            out=wv[:, :, ci, :],
            in_=ins["wv" + sfx].rearrange("(kt p) j -> p kt j", p=128))

    def emit_qk(wname, pair, ctab, stab):
        """Load one pair's weight slices, run the projection, apply RoPE."""
        w_re = pw.tile([128, KT, 128], MM_DT, name=f"{wname}{pair}_re", tag="w_re")
        w_im = pw.tile([128, KT, 128], MM_DT, name=f"{wname}{pair}_im", tag="w_im")
        w_imn = pw.tile([128, KT, 128], MM_DT, name=f"{wname}{pair}_imn", tag="w_imn")
        for wt, sfx in ((w_re, "T_re"), (w_im, "T_im"), (w_imn, "T_imn")):
            src = ins[wname + sfx].rearrange(
                "(kt p) (pair j) -> p kt pair j", p=128, pair=2)
            nc.gpsimd.dma_start(out=wt, in_=src[:, :, pair, :])
        for st in range(2):
            ssl = slice(st * 512, st * 512 + 512)
            ps_r = pmm.tile([128, 512], F32, name="ps_r", tag="mm")
            ps_i = pmm.tile([128, 512], F32, name="ps_i", tag="mm")
            for kt in range(KT):
                xr = x_re[:, kt, ssl]
                xi = x_im[:, kt, ssl]
                nc.tensor.matmul(ps_r, lhsT=w_re[:, kt, :], rhs=xr,
                                 start=(kt == 0), stop=False)
                nc.tensor.matmul(ps_i, lhsT=w_re[:, kt, :], rhs=xi,
                                 start=(kt == 0), stop=False)
                nc.tensor.matmul(ps_r, lhsT=w_imn[:, kt, :], rhs=xi,
                                 start=False, stop=(kt == KT - 1))
                nc.tensor.matmul(ps_i, lhsT=w_im[:, kt, :], rhs=xr,
                                 start=False, stop=(kt == KT - 1))
            # RoPE products: p1=Tr*c p2=Ti*s p3=Tr*s p4=Ti*c
            ct = tabs[ctab][:, ssl]
            st_t = tabs[stab][:, ssl]
            p1 = pt.tile([128, 512], F32, name="p1", tag="ropetmp")
            p2 = pt.tile([128, 512], F32, name="p2", tag="ropetmp")
            p3 = pt.tile([128, 512], F32, name="p3", tag="ropetmp")
            p4 = pt.tile([128, 512], F32, name="p4", tag="ropetmp")
            nc.vector.tensor_mul(p1, ps_r, ct)
            nc.vector.tensor_mul(p2, ps_i, st_t)
            nc.vector.tensor_mul(p3, ps_r, st_t)
            nc.vector.tensor_mul(p4, ps_i, ct)
            if wname == "wq":
                nc.vector.tensor_sub(q_r[:, pair, ssl], p1, p2)
                nc.vector.scalar_tensor_tensor(
                    q_in[:, pair, ssl], in0=p3, scalar=-1.0, in1=p4,
                    op0=ALU.mult, op1=ALU.subtract)
            else:
                nc.vector.tensor_sub(k_r[:, pair, ssl], p1, p2)
                nc.vector.tensor_sub(k_rn[:, pair, ssl], p2, p1)
                nc.vector.scalar_tensor_tensor(
                    k_in[:, pair, ssl], in0=p3, scalar=-1.0, in1=p4,
                    op0=ALU.mult, op1=ALU.subtract)

    def emit_v():
        for tblk in range(KT):
            ps_v = pmm.tile([128, 512], F32, name="ps_v", tag="mm")
            for kt in range(KT):
                lx_re = x_re[:, kt, tblk * 128:(tblk + 1) * 128]
                lx_im = x_im[:, kt, tblk * 128:(tblk + 1) * 128]
                nc.tensor.matmul(ps_v, lhsT=lx_re, rhs=wv[:, kt, 1:3, :],
                                 start=(kt == 0), stop=False)
                nc.tensor.matmul(ps_v, lhsT=lx_im, rhs=wv[:, kt, 0:2, :],
                                 start=False, stop=(kt == KT - 1))
            # copy out (ACT): v comps (0: -Vi, 1: Vr, 2: Vi)
            nc.scalar.copy(v[:, tblk, 1, :], ps_v[:, 0:256])
            nc.scalar.copy(v[:, tblk, 2, :], ps_v[:, 256:512])
            nc.scalar.activation(v[:, tblk, 0, :], ps_v[:, 256:512],
                                 ACTF.Copy, scale=-1.0)

    state = {"prev_last_sin": None}

    def emit_exp(pair, sh, exp_insts):
        """Scores + exp + range-wraps + denominator for one chunk.

        Per-head data is packed at [:, hh*512:(hh+1)*512] of [128, 1024]
        SBUF tiles so downstream sins/muls run at N=1024.
        """
        ssl = slice(sh * 512, sh * 512 + 512)
        E_tiles, W_tiles, WC_tiles = [], [], []
        r_ps = ppr.tile([128, 512], F32, name="r_ps", tag="ps_r")
        for tblk in range(KT):
            tsl = slice(tblk * 128, tblk * 128 + 128)
            pss = {}
            for hh in range(2):
                pss[hh] = (
                    pmm.tile([128, 512], F32, name="ps_sre", tag="mm"),
                    pmm.tile([128, 512], F32, name="ps_sip", tag="mm"),
                )
            # interleave hh so the two k=64 row-groups overlap on PE
            for chain in range(4):
                for hh in range(2):
                    dsl = slice(hh * 64, hh * 64 + 64)
                    ps_re, ps_ip = pss[hh]
                    lkr = k_r[dsl, pair, tsl]
                    lkin = k_in[dsl, pair, tsl]
                    lkrn = k_rn[dsl, pair, tsl]
                    rqr = q_r[dsl, pair, ssl]
                    rqin = q_in[dsl, pair, ssl]
                    if chain == 0:
                        nc.tensor.matmul(ps_re, lhsT=lkr, rhs=rqr,
                                         start=True, stop=False)
                    elif chain == 1:
                        nc.tensor.matmul(ps_ip, lhsT=lkrn, rhs=rqin,
                                         start=True, stop=False)
                    elif chain == 2:
                        nc.tensor.matmul(ps_re, lhsT=lkin, rhs=rqin,
                                         start=False, stop=True)
                    else:
                        nc.tensor.matmul(ps_ip, lhsT=lkin, rhs=rqr,
                                         start=False, stop=True)
            Eb = pE.tile([128, 1024], MM_DT, name="Eb", tag="E")
            Wb = pW.tile([128, 1024], MM_DT, name="Wb", tag="W")
            WCb = pWC.tile([128, 1024], MM_DT, name="WCb", tag="WC")
            for hh in range(2):
                hsl = slice(hh * 512, hh * 512 + 512)
                ps_re, ps_ip = pss[hh]
                ei = nc.scalar.activation(Eb[:, hsl], ps_re, ACTF.Exp)
                exp_insts.append(ei)
                if state["prev_last_sin"] is not None:
                    _dep(state["prev_last_sin"], ei, "act-table: exp after sins")
                nc.tensor.matmul(r_ps[hh * 64:hh * 64 + 1, :],
                                 lhsT=ones_col, rhs=Eb[:, hsl],
                                 start=(tblk == 0), stop=(tblk == KT - 1))
                nc.vector.add_range_wrap(Wb[:, hsl], ps_ip, shift=0.0,
                                         bound=PI, period=2.0 * PI)
                nc.vector.add_range_wrap(WCb[:, hsl], ps_ip, shift=PI / 2.0,
                                         bound=PI, period=2.0 * PI)
            E_tiles.append(Eb)
            W_tiles.append(Wb)
            WC_tiles.append(WCb)
        # denominators -> broadcast -> fast reciprocal (still exp-block)
        Rb = {}
        for hh in range(2):
            rrow = psm.tile([1, 512], F32, name="rrow", tag="rrow")
            nc.vector.tensor_copy(rrow, r_ps[hh * 64:hh * 64 + 1, :])
            rbraw = psm.tile([64, 512], F32, name="rbraw", tag="rbraw")
            nc.gpsimd.partition_broadcast(rbraw, rrow)
            rb = psm.tile([64, 512], F32, name="rb", tag="rb")
            nc.vector.reciprocal_approx_fast(rb, rbraw)
            Rb[hh] = rb
        return (E_tiles, W_tiles, WC_tiles, Rb)

    def emit_sin(pair, sh, chunk_state, last_exp):
        ssl = slice(sh * 512, sh * 512 + 512)
        E_tiles, W_tiles, WC_tiles, Rb = chunk_state
        at_re = pat.tile([128, 512], F32, name="at_re", tag="ps_at")
        at_im = pat.tile([128, 512], F32, name="at_im", tag="ps_at")
        for tblk in range(KT):
            cw = pbt.tile([128, 1024], MM_DT, name="cw", tag="sintmp")
            si1 = nc.scalar.activation(cw, WC_tiles[tblk], ACTF.Sin)  # cos(im)
            sw = pbt.tile([128, 1024], MM_DT, name="sw", tag="sintmp")
            si2 = nc.scalar.activation(sw, W_tiles[tblk], ACTF.Sin)   # sin(im)
            _dep(last_exp, si1, "act-table: sins after exps")
            _dep(last_exp, si2, "act-table: sins after exps")
            state["prev_last_sin"] = si2
            ar = pbt.tile([128, 1024], MM_DT, name="ar", tag="avr")
            nc.vector.tensor_mul(ar, E_tiles[tblk], cw)         # exp*cos
            ai = pbt.tile([128, 1024], MM_DT, name="ai", tag="avr")
            nc.vector.tensor_mul(ai, E_tiles[tblk], sw)         # exp*sin
            # AV matmuls, hh-interleaved for col-group overlap
            for chain in range(4):
                for hh in range(2):
                    hsl = slice(hh * 512, hh * 512 + 512)
                    jsl = slice(pair * 128 + hh * 64, pair * 128 + hh * 64 + 64)
                    psl = slice(hh * 64, hh * 64 + 64)
                    lvin = v[:, tblk, 0, jsl]
                    lvr = v[:, tblk, 1, jsl]
                    lvi = v[:, tblk, 2, jsl]
                    if chain == 0:
                        nc.tensor.matmul(at_re[psl, :], lhsT=lvr, rhs=ar[:, hsl],
                                         start=(tblk == 0), stop=False)
                    elif chain == 1:
                        nc.tensor.matmul(at_im[psl, :], lhsT=lvi, rhs=ar[:, hsl],
                                         start=(tblk == 0), stop=False)
                    elif chain == 2:
                        nc.tensor.matmul(at_re[psl, :], lhsT=lvin, rhs=ai[:, hsl],
                                         start=False, stop=(tblk == KT - 1))
                    else:
                        nc.tensor.matmul(at_im[psl, :], lhsT=lvr, rhs=ai[:, hsl],
                                         start=False, stop=(tblk == KT - 1))
        # normalize + copy out
        for hh in range(2):
            psl = slice(hh * 64, hh * 64 + 64)
            jj = slice(hh * 64, hh * 64 + 64)
            nc.vector.tensor_mul(attn_re[jj, pair, ssl], at_re[psl, :], Rb[hh])
            nc.vector.tensor_mul(attn_im[jj, pair, ssl], at_im[psl, :], Rb[hh])

    # =================== pipelined emission ===================
    # Per-chunk exp/sin table phases (E00 S00 E01 S01 ...) — the E-tile pool
    # only holds one chunk, so a later chunk's exps must come after the
    # previous chunk's sins anyway (slot reuse); the scheduler overlaps the
    # next chunk's score matmuls with the current sin-block on the PE.
    emit_qk("wq", 0, "qc8", "qs8")
    emit_qk("wk", 0, "kcos", "ksin")
    exps00 = []
    st00 = emit_exp(0, 0, exps00)
    emit_qk("wq", 1, "qc8", "qs8")
    emit_qk("wk", 1, "kcos", "ksin")
    emit_v()
    phA.close()  # release x/weights/tables space before phase C pools open

    # phase-C pools take over phase A's space
    pc_ctx = ExitStack()
    pc = pc_ctx.enter_context(tc.tile_pool(name="phC", bufs=1))
    po = pc_ctx.enter_context(tc.tile_pool(name="phC_o", bufs=4))
    wo = {}
    for sfx in ("T_re", "T_im", "T_imn"):
        wt = pc.tile([128, 2, S], MM_DT, name=f"wo{sfx}", tag=f"wo{sfx}")
        nc.sync.dma_start(
            out=wt, in_=ins["wo" + sfx].rearrange("(kt p) n -> p kt n", p=128))
        wo[sfx] = wt

    emit_sin(0, 0, st00, exps00[-1])
    for pair, sh in ((0, 1), (1, 0), (1, 1)):
        exps = []
        st = emit_exp(pair, sh, exps)
        emit_sin(pair, sh, st, exps[-1])

    # =================== Phase C: output projection ===================
    for sblk in range(KT):
        bsl = slice(sblk * 128, sblk * 128 + 128)
        for nt in range(2):
            nsl = slice(nt * 512, nt * 512 + 512)
            ps_or = pmm.tile([128, 512], F32, name="ps_or", tag="mm")
            ps_oi = pmm.tile([128, 512], F32, name="ps_oi", tag="mm")
            for kt in range(2):  # contraction over j (= pair dim)
                la_r = attn_re[:, kt, bsl]
                la_i = attn_im[:, kt, bsl]
                nc.tensor.matmul(ps_or, lhsT=la_r, rhs=wo["T_re"][:, kt, nsl],
                                 start=(kt == 0), stop=False)
                nc.tensor.matmul(ps_oi, lhsT=la_r, rhs=wo["T_im"][:, kt, nsl],
                                 start=(kt == 0), stop=False)
                nc.tensor.matmul(ps_or, lhsT=la_i, rhs=wo["T_imn"][:, kt, nsl],
                                 start=False, stop=(kt == 1))
                nc.tensor.matmul(ps_oi, lhsT=la_i, rhs=wo["T_re"][:, kt, nsl],
                                 start=False, stop=(kt == 1))
            o_r = po.tile([128, 512], F32, name="o_r", tag="otmp")
            o_i = po.tile([128, 512], F32, name="o_i", tag="otmp")
            nc.vector.tensor_copy(o_r, ps_or)
            nc.vector.tensor_copy(o_i, ps_oi)
            nc.sync.dma_start(out=outs["out_re"][bsl, nsl], in_=o_r)
            nc.sync.dma_start(out=outs["out_im"][bsl, nsl], in_=o_i)

    pc_ctx.close()
    pB.close()
    ctx.close()


_IN_SPECS = (
    [("xT_re", [D, S], MM_DT), ("xT_im", [D, S], MM_DT)]
    + [(w + sfx, [D, JG], MM_DT) for w in ("wq", "wk", "wv")
       for sfx in ("T_re", "T_im", "T_imn")]
    + [("wo" + sfx, [JG, D], MM_DT) for sfx in ("T_re", "T_im", "T_imn")]
    + [(t, [128, S], TAB_DT) for t in ("qc8", "qs8", "kcos", "ksin")]
)


def build_program():
    nc = bacc.Bacc("TRN2", target_bir_lowering=False, debug=False,
                   enable_asserts=False, num_devices=8)
    ins = {name: nc.dram_tensor(name, shape, dt, kind="ExternalInput").ap()
           for name, shape, dt in _IN_SPECS}
    outs = {name: nc.dram_tensor(name, [S, D], F32, kind="ExternalOutput").ap()
            for name in ("out_re", "out_im")}
    with tile.TileContext(nc) as tc:
        _build_kernel(tc, ins, outs)
    nc.compile()
    return nc


def _make_tables():
    inv_freq = 1.0 / (10000.0 ** (np.arange(DH, dtype=np.float64) / DH))
    ang = np.arange(S, dtype=np.float64)[:, None] * inv_freq[None, :]  # [S, DH]
    angT = ang.T  # [DH, S]
    ang128 = np.concatenate([angT, angT], axis=0)  # [128, S]
    c = np.cos(ang128)
    s = np.sin(ang128)
    tab_np = np.float16
    return {
        "qc8": (c * 0.125).astype(tab_np),
        "qs8": (s * 0.125).astype(tab_np),
        "kcos": c.astype(tab_np),
        "ksin": s.astype(tab_np),
    }


def _core_inputs(inputs, c, tables):
    b, g = divmod(c, 4)
    rows = slice(g * JG, (g + 1) * JG)

    def f(a):
        return np.ascontiguousarray(np.asarray(a, dtype=np.float32)).astype(MM_NP)

    m = {
        "xT_re": f(np.asarray(inputs["x_re"])[b].T),
        "xT_im": f(np.asarray(inputs["x_im"])[b].T),
        "woT_re": f(np.asarray(inputs["wo_re"])[:, rows].T),
        "woT_im": f(np.asarray(inputs["wo_im"])[:, rows].T),
        "woT_imn": f(-np.asarray(inputs["wo_im"])[:, rows].T),
    }
    for w in ("wq", "wk", "wv"):
        wre = np.asarray(inputs[w + "_re"])[rows]
        wim = np.asarray(inputs[w + "_im"])[rows]
        m[w + "T_re"] = f(wre.T)
        m[w + "T_im"] = f(wim.T)
        m[w + "T_imn"] = f(-wim.T)
    m.update(tables)
    return m


_PROGRAM = None


def _get_program():
    global _PROGRAM
    if _PROGRAM is None:
        _PROGRAM = build_program()
    return _PROGRAM


def run(inputs, trace=False, **kwargs):
    nc = _get_program()
    tables = _make_tables()
    in_maps = [_core_inputs(inputs, c, tables) for c in range(8)]
    res = run_bass_kernel_spmd(nc, in_maps, list(range(8)), trace=trace, **kwargs)
    B = 2
    out = np.zeros((B, S, D, 2), np.float32)
    for c, r in enumerate(res.results):
        b = c // 4
        out[b, :, :, 0] += r["out_re"]
        out[b, :, :, 1] += r["out_im"]
    return out, res


def kernel(**inputs):
    out, _ = run(inputs)
    return out


if __name__ == "__main__":
    nc = build_program()
    print("program built + compiled OK")
